# revision 18
# baseline (speedup 1.0000x reference)
"""Multi-head attention (B=4, S=2048, D=1024, H=16, Hd=64) on 8 trn2 cores.

Sharding: core c = (batch b = c // 2, head-group hg = c % 2). Each core
computes attention for 8 heads of one batch and the corresponding slice of
the output projection; host sums the two partial outputs per batch.

Schedule model (per core, all matmuls bf16 with fp32 PSUM accumulation):
  PE pure work ~285us, ScalarE exp ~275us (256 x [128,1024] exps incl
  per-inst sync) -- both engines must stay saturated and overlapped.
  - DMA is issued column-blocked (xt by 512-wide q blocks interleaved with
    the weight slices the prologue needs first) so the first projection
    matmul starts ~3us in, not after the full 4MB xt load.
  - Prologue: kT p0 (4 groups) + qT(p0,c0)/(p1,c0); everything else (kT
    p1-p3, qT, v half1, o-proj) is filler work drained adaptively: every
    chunk iteration force-emits deadline-due fillers and then spends a
    budget of remaining_filler_cost/remaining_iterations, so the PE never
    starves while ScalarE paces the exp stream.
  - v projection is computed in half-groups (heads 0-3 / 4-7, N=256):
    half0 just-in-time inside chunk 0 (attnV[i] consumes v[st=i]), half1
    via fillers before the first p2 chunk.
  - Scores pair per (i): two K=64 matmuls on disjoint PE row halves run
    concurrently (~216ns/pair); attnV rides the denominator as a 65th
    output row; normalization is deferred one chunk and applied via K=1
    ones-matmul broadcasts + a DVE multiply.
  - PSUM: dedicated pools -- stt (scores+exp) 2 x [128,1024] (4 banks),
    otA/otB 2 x [65,512] (2 banks), proj/norm 2 x [128,512] (2 banks).

Per-core layout:
  xt   = x[b].T                    [D=1024, S=2048]  (lhsT/rhs K-major)
  qT/kT = (Wslice.T @ ..)          [512, 2048]  d-major, 4 pair-tiles of 128
  v    = x @ Wv_slice              [2048, 512]  s-major (+ones col per head)
  per head-pair chunk: scoresT[k,q] (row-tiled K=64 pair) -> exp
            outT[d,q] += v-block.T @ expT ; denom[q] += ones row
  y = outT.T-blocks @ Wo_slice + bo   [2048, 1024] bf16 partial
"""

import numpy as np
import ml_dtypes

S = 2048
D = 1024
HG_D = 512          # head dims per core (8 heads x 64)
NH = 8              # heads per core
KT = S // 128       # 16 k-tiles
DT = D // 128       # 8 contraction tiles for QKV
ST = S // 128       # 16 s-tiles
OT = HG_D // 128    # 4 contraction tiles for O-proj / pair tiles
N_CORES = 8

BF16 = ml_dtypes.bfloat16

_CACHED_NC = {}

# Chunk order (p = head-pair tile, c = 512-wide q chunk): all p0/p1 chunks
# first, then p2/p3 — so only v-half0 + kT p0/p1 are needed early and the
# p2/p3 prerequisites (v-half1, kT p2/p3) spread as budget fillers across
# eight chunks instead of being deadline-crammed into chunks 1-3.
CHUNK_ORDER = ([(p, c) for c in range(4) for p in (0, 1)]
               + [(p, c) for c in range(4) for p in (2, 3)])


def _build_nc(with_bq=True, with_bk=True, with_bv=True, with_bo=True):
    import concourse.bass as bass  # noqa: F401
    import concourse.mybir as mybir
    import concourse.tile as tile
    from concourse import bacc

    f32 = mybir.dt.float32
    bf16 = mybir.dt.bfloat16
    Exp = mybir.ActivationFunctionType.Exp

    nc = bacc.Bacc("TRN2", target_bir_lowering=False, debug=False,
                   num_devices=N_CORES)

    xt_d = nc.dram_tensor("xt", [D, S], bf16, kind="ExternalInput")
    wq_d = nc.dram_tensor("wq", [D, HG_D], bf16, kind="ExternalInput")
    wk_d = nc.dram_tensor("wk", [D, HG_D], bf16, kind="ExternalInput")
    wv_d = nc.dram_tensor("wv", [D, HG_D], bf16, kind="ExternalInput")
    wo_d = nc.dram_tensor("wo", [HG_D, D], bf16, kind="ExternalInput")
    bq_d = nc.dram_tensor("bqt", [128, OT], f32, kind="ExternalInput")
    bk_d = nc.dram_tensor("bkt", [128, OT], f32, kind="ExternalInput")
    bv_d = nc.dram_tensor("bvr", [1, HG_D], bf16, kind="ExternalInput")
    bo_d = nc.dram_tensor("bor", [1, D], bf16, kind="ExternalInput")
    y_d = nc.dram_tensor("y", [S, D], bf16, kind="ExternalOutput")

    with tile.TileContext(nc) as tc:
        with (
            tc.tile_pool(name="cpool", bufs=1) as cpool,
            tc.tile_pool(name="wpool", bufs=2) as wpool,
            tc.tile_pool(name="sttpool", bufs=2, space="PSUM") as sttpool,
            tc.tile_pool(name="pjpool", bufs=2, space="PSUM") as pjpool,
            tc.tile_pool(name="popool", bufs=2, space="PSUM") as popool,
        ):
            # ---- persistent SBUF tiles ----
            xt_sb = cpool.tile([128, DT, S], bf16, name="xt_sb")
            wq_sb = cpool.tile([128, DT, HG_D], bf16, name="wq_sb")
            wk_sb = cpool.tile([128, DT, HG_D], bf16, name="wk_sb")
            wv_sb = cpool.tile([128, DT, HG_D], bf16, name="wv_sb")
            wo_sb = cpool.tile([128, OT, D], bf16, name="wo_sb")
            bq_sb = cpool.tile([128, OT], f32, name="bq_sb")
            bk_sb = cpool.tile([128, OT], f32, name="bk_sb")
            bvr_sb = cpool.tile([1, HG_D], bf16, name="bvr_sb")
            bor_sb = cpool.tile([1, D], bf16, name="bor_sb")
            ones_t = cpool.tile([128, 128], bf16, name="ones_t")
            qT_sb = cpool.tile([128, OT, S], bf16, name="qT_sb")
            kT_sb = cpool.tile([128, OT, S], bf16, name="kT_sb")
            # v with a trailing ones column per head: attnv lhsT [128, 65]
            # whose 65th output row accumulates the softmax denominator.
            v_sb = cpool.tile([128, ST, NH, 65], bf16, name="v_sb")
            aoT_sb = cpool.tile([128, OT, S], bf16, name="aoT_sb")

            # ---- loads ----
            # One multi-dim descriptor per tensor piece (descriptor issue on
            # the Sync queue costs ~650ns each, so batching matters), ordered
            # so the prologue's first consumers unblock earliest: the first
            # kT group needs wk + xt q-block 0 only.
            xt_r = xt_d.rearrange("(k p) s -> p k s", p=128)
            nc.sync.dma_start(out=wk_sb[:], in_=wk_d.rearrange(
                "(k p) c -> p k c", p=128))
            nc.sync.dma_start(out=xt_sb[:, :, 0:512], in_=xt_r[:, :, 0:512])
            nc.sync.dma_start(out=wq_sb[:], in_=wq_d.rearrange(
                "(k p) c -> p k c", p=128))
            nc.sync.dma_start(out=wv_sb[:], in_=wv_d.rearrange(
                "(k p) c -> p k c", p=128))
            nc.sync.dma_start(out=xt_sb[:, :, 512:1024],
                              in_=xt_r[:, :, 512:1024])
            nc.sync.dma_start(out=xt_sb[:, :, 1024:2048],
                              in_=xt_r[:, :, 1024:2048])
            nc.sync.dma_start(out=wo_sb[:], in_=wo_d.rearrange(
                "(k p) c -> p k c", p=128))
            if with_bq:
                nc.sync.dma_start(out=bq_sb[:], in_=bq_d[:])
            if with_bk:
                nc.sync.dma_start(out=bk_sb[:], in_=bk_d[:])
            if with_bv:
                nc.sync.dma_start(out=bvr_sb[:], in_=bv_d[:])
            if with_bo:
                nc.sync.dma_start(out=bor_sb[:], in_=bo_d[:])
            nc.gpsimd.memset(ones_t[:], 1.0)
            # only the per-head trailing ones-column needs the constant
            nc.vector.memset(v_sb[:, :, :, 64:65], 1.0)

            # ---- projection group emitters ----
            def emit_qk_group(which, p, jc, on_scalar, half=None):
                if which == "q":
                    w_sb, b_sb, out_sb, wb = wq_sb, bq_sb, qT_sb, with_bq
                else:
                    w_sb, b_sb, out_sb, wb = wk_sb, bk_sb, kT_sb, with_bk
                lo = jc * 512 + (0 if not half else 256)
                w = 512 if half is None else 256
                pq = pjpool.tile([128, w], f32, tag="ps", name="pq")
                for k in range(DT):
                    nc.tensor.matmul(
                        pq[:],
                        w_sb[:, k, p * 128:(p + 1) * 128],
                        xt_sb[:, k, lo:lo + w],
                        start=(k == 0), stop=(k == DT - 1),
                    )
                dst = out_sb[:, p, lo:lo + w]
                if wb:
                    nc.scalar.add(dst, pq[:], b_sb[:, p:p + 1])
                elif on_scalar:
                    nc.scalar.copy(dst, pq[:])
                else:
                    nc.vector.tensor_copy(dst, pq[:])

            def emit_v_half(st, half):
                # heads 4*half..4*half+3, i.e. wv cols [256*half, 256*half+256)
                lo = 256 * half
                pv = pjpool.tile([128, 256], f32, tag="ps", name="pv")
                for k in range(DT):
                    nc.tensor.matmul(
                        pv[:],
                        xt_sb[:, k, st * 128:(st + 1) * 128],
                        wv_sb[:, k, lo:lo + 256],
                        start=(k == 0), stop=(not with_bv and k == DT - 1),
                    )
                if with_bv:
                    nc.tensor.matmul(pv[:], ones_t[0:1, 0:128],
                                     bvr_sb[0:1, lo:lo + 256],
                                     start=False, stop=True)
                nc.vector.tensor_copy(
                    v_sb[:, st, 4 * half:4 * half + 4, 0:64],
                    pv.rearrange("p (h c) -> p h c", c=64))

            def emit_oproj_half(st, l, on_scalar=False):
                yt = wpool.tile([128, 512], bf16, tag="y", bufs=3, name="yt")
                py = pjpool.tile([128, 512], f32, tag="ps", name="py")
                for kt in range(OT):
                    nc.tensor.matmul(
                        py[:],
                        aoT_sb[:, kt, st * 128:(st + 1) * 128],
                        wo_sb[:, kt, l * 512:(l + 1) * 512],
                        start=(kt == 0),
                        stop=(not with_bo and kt == OT - 1),
                    )
                if with_bo:
                    nc.tensor.matmul(py[:], ones_t[0:1, 0:128],
                                     bor_sb[0:1, l * 512:(l + 1) * 512],
                                     start=False, stop=True)
                if on_scalar:
                    nc.scalar.copy(yt[:], py[:])
                else:
                    nc.vector.tensor_copy(yt[:], py[:])
                nc.sync.dma_start(
                    out=y_d[st * 128:(st + 1) * 128, l * 512:(l + 1) * 512],
                    in_=yt[:])

            # ---- scheduling priority bands ----
            # Band 0: scores+exp (the exp cadence is the kernel clock).
            # Band 1: attnV + JIT v halves.
            # Band 2: fillers (kT/qT/v prereqs, o-proj backfill, norm) —
            #         above the chunk-end drain so a due filler's PSUM->SBUF
            #         copy isn't stuck behind the aoT/den/recip chain.
            # Band 3: chunk-end drain (aoT casts, denominator, reciprocal).
            import contextlib

            band_counters = [0, 10_000_000, 20_000_000, 30_000_000]

            @contextlib.contextmanager
            def band(n):
                saved = tc.cur_priority
                tc.cur_priority = band_counters[n]
                try:
                    yield
                finally:
                    band_counters[n] = tc.cur_priority
                    tc.cur_priority = saved

            # ---- deferred normalization ----
            pending = []
            norm_done = {c: 0 for c in range(4)}
            oproj_emitted = set()

            # ---- filler queue: (deadline, cost_ns, item) in deadline order.
            # item kinds: ("k",p,jc,half) ("q",p,jc,half) ("v",st,half)
            #             ("o",st,l)
            idx_of = {pc: i for i, pc in enumerate(CHUNK_ORDER)}
            first_p_idx = {p: min(i for (pp, _), i in idx_of.items()
                                  if pp == p) for p in range(4)}
            fillers = []
            for p in (1, 2, 3):
                for jc in range(4):
                    if p == 1 and jc < 2:
                        continue  # in prologue
                    for h in (0, 1):
                        fillers.append((first_p_idx[p] + jc / 4, 872,
                                        ("k", p, jc, h)))
            v1_dl = first_p_idx[2]
            for st in range(ST):
                fillers.append((v1_dl + st / 16, 872, ("v", st, 1)))
            for idx, (p, c) in enumerate(CHUNK_ORDER):
                if idx >= 2:
                    for h in (0, 1):
                        fillers.append((idx - 0.3, 872, ("q", p, c, h)))
            fillers.sort(key=lambda f: f[0])
            rem_cost = [sum(f[1] for f in fillers)]

            def emit_filler(item):
                kind = item[0]
                with band(2):
                    if kind == "k" or kind == "q":
                        emit_qk_group(item[0], item[1], item[2],
                                      on_scalar=False, half=item[3])
                    elif kind == "v":
                        emit_v_half(item[1], item[2])
                    else:
                        emit_oproj_half(item[1], item[2])

            def add_oproj(c):
                for st in range(4 * c, 4 * c + 4):
                    if st not in oproj_emitted:
                        oproj_emitted.add(st)
                        for l in range(2):
                            fillers.append((float("inf"), 864, ("o", st, l)))
                            rem_cost[0] += 864

            def flush_normalize():
                with band(2):
                    _flush_normalize()

            def _flush_normalize():
                off2, p2, c2, rb2, row = pending.pop(0)
                off3, p3, c3, rb3, row3 = pending.pop(0)
                assert p2 == p3 and c2 == c3
                jb = slice(c2 * 512, (c2 + 1) * 512)
                bt = pjpool.tile([128, 512], f32, tag="ps", name="bt")
                nc.tensor.matmul(
                    bt[off2:off2 + 64, :],
                    ones_t[row:row + 1, 0:64],
                    rb2[row:row + 1, :],
                    start=True, stop=True,
                )
                nc.tensor.matmul(
                    bt[off3:off3 + 64, :],
                    ones_t[row3:row3 + 1, 0:64],
                    rb3[row3:row3 + 1, :],
                    start=True, stop=True,
                )
                nc.vector.tensor_mul(
                    aoT_sb[:, p2, jb],
                    aoT_sb[:, p2, jb],
                    bt[:])
                norm_done[c2] += 1
                if norm_done[c2] == OT:
                    add_oproj(c2)

            # drain fillers: everything past-due first (those are needed by
            # this very iteration's consumers), then optional budget work,
            # paced by remaining supply and capped by the iteration's PE
            # capacity so work never bunches. Forced items don't poison the
            # budget accumulator.
            iters_left = [len(CHUNK_ORDER) * KT]
            debt = [0.0]
            ITER_CAP = 1150.0

            def drain_fillers(now, base):
                work = base
                while fillers and fillers[0][0] <= now:
                    _, cost, item = fillers.pop(0)
                    rem_cost[0] -= cost
                    work += cost
                    emit_filler(item)
                debt[0] = min(debt[0] + rem_cost[0] / max(iters_left[0], 1),
                              1744.0)
                while (fillers and debt[0] > 0
                       and work + fillers[0][1] <= ITER_CAP + 400):
                    _, cost, item = fillers.pop(0)
                    rem_cost[0] -= cost
                    debt[0] -= cost
                    work += cost
                    emit_filler(item)

            # ---- attention chunk ----
            def emit_chunk(idx, p, c):
                jb = slice(c * 512, (c + 1) * 512)
                otA = popool.tile([65, 512], f32, tag="po", name="otA")
                otB = popool.tile([65, 512], f32, tag="po", name="otB")
                for i in range(KT):
                    # fillers FIRST: deadline-due producers (kT/qT/v read by
                    # this very iteration) must precede their consumers in
                    # program order, plus the adaptive budget share.
                    drain_fillers(idx + i / KT,
                                  base=647.0 + (872.0 if idx == 0 else 0.0))
                    if idx == 0:
                        with band(1):
                            emit_v_half(i, 0)
                    with band(0):
                        stt = sttpool.tile([128, 1024], f32, tag="stt",
                                           name="stt")
                        for off in (0, 64):
                            nc.tensor.matmul(
                                stt[:, off * 8:off * 8 + 512],
                                kT_sb[off:off + 64, p, i * 128:(i + 1) * 128],
                                qT_sb[off:off + 64, p, jb],
                                start=True, stop=True,
                            )
                        et = wpool.tile([128, 1024], bf16, tag="exp", bufs=8,
                                        name="et")
                        nc.scalar.activation(et[:], stt[:], Exp, scale=0.125)
                    with band(1):
                        for ot, hh in ((otA, 0), (otB, 1)):
                            nc.tensor.matmul(
                                ot[:],
                                v_sb[:, i, 2 * p + hh, :],
                                et[:, hh * 512:(hh + 1) * 512],
                                start=(i == 0), stop=(i == KT - 1),
                            )
                    if i == 15:
                        # flush the previous chunk's pair (its reciprocal
                        # finished ~a chunk ago) so o-proj unlocks early
                        while len(pending) > 0:
                            flush_normalize()
                    iters_left[0] -= 1
                # Drain attn rows first so the PSUM tiles free fast, then
                # gather the denominator rows and batch-reciprocal. Band 3:
                # this chain must not block due fillers' PSUM->SBUF copies
                # on the Vector queue at chunk boundaries.
                ctx = band(3)
                ctx.__enter__()
                nc.vector.tensor_copy(aoT_sb[0:64, p, jb], otA[0:64, :])
                nc.vector.tensor_copy(aoT_sb[64:128, p, jb], otB[0:64, :])
                den = wpool.tile([33, 512], f32, tag="den", bufs=2,
                                 name="den")
                nc.vector.tensor_copy(den[0:1, :], otA[64:65, :])
                nc.vector.tensor_copy(den[32:33, :], otB[64:65, :])
                rf = wpool.tile([33, 512], f32, tag="rf", name="rf")
                rb = wpool.tile([65, 512], bf16, tag="rb", bufs=3,
                                name="rb")
                # approx_fast needs a partition-0-based AP; rows 1..31 hold
                # stale values (harmless, unread). ~5x faster than
                # reciprocal(), ~3e-6 rel err.
                nc.vector.reciprocal_approx_fast(rf[0:33, :], den[0:33, :])
                nc.vector.tensor_copy(rb[64:65, :], rf[0:1, :])
                nc.vector.tensor_copy(rb[32:33, :], rf[32:33, :])
                pending.append((0, p, c, rb, 64))
                pending.append((64, p, c, rb, 32))
                ctx.__exit__(None, None, None)

            # ---- prologue (ScalarE is idle before the first exp) ----
            # qT right after the first kT group: when the xt DMA completes it
            # unblocks everything at once and the scheduler follows emission
            # order, so the first chunk's q must come before the other kTs.
            # kT p1 jc0/jc1 ride here too (the prologue is DMA-paced and has
            # PE slack), halving chunk 1's deadline burst.
            emit_qk_group("k", 0, 0, on_scalar=True)
            emit_qk_group("q", *CHUNK_ORDER[0], on_scalar=True)
            emit_qk_group("q", *CHUNK_ORDER[1], on_scalar=True)
            for jc in range(1, 4):
                emit_qk_group("k", 0, jc, on_scalar=True)
            emit_qk_group("k", 1, 0, on_scalar=True)
            emit_qk_group("k", 1, 1, on_scalar=True)

            # ---- main schedule ----
            for idx, (p, c) in enumerate(CHUNK_ORDER):
                emit_chunk(idx, p, c)

            # ---- tail: flush the last normalizations + remaining o-proj ----
            while pending:
                flush_normalize()
            n = 0
            rest = [f[2] for f in fillers]
            fillers.clear()
            for item in rest:
                if item[0] == "o":
                    emit_oproj_half(item[1], item[2], on_scalar=(n % 2 == 1))
                    n += 1
                else:
                    emit_filler(item)
            for st in range(ST):
                if st not in oproj_emitted:
                    oproj_emitted.add(st)
                    for l in range(2):
                        emit_oproj_half(st, l, on_scalar=(n % 2 == 1))
                        n += 1

    nc.compile()
    return nc


def get_nc(with_bq=True, with_bk=True, with_bv=True, with_bo=True):
    key = (with_bq, with_bk, with_bv, with_bo)
    if key not in _CACHED_NC:
        _CACHED_NC[key] = _build_nc(*key)
    return _CACHED_NC[key]


def make_in_maps(x, Wq, bq, Wk, bk, Wv, bv, Wo, bo):
    x = np.asarray(x, dtype=np.float32)
    in_maps = []
    for c in range(N_CORES):
        b, hg = c // 2, c % 2
        sl = slice(hg * HG_D, (hg + 1) * HG_D)
        in_maps.append({
            "xt": np.ascontiguousarray(np.asarray(x[b]).T).astype(BF16),
            "wq": np.ascontiguousarray(np.asarray(Wq)[:, sl]).astype(BF16),
            "wk": np.ascontiguousarray(np.asarray(Wk)[:, sl]).astype(BF16),
            "wv": np.ascontiguousarray(np.asarray(Wv)[:, sl]).astype(BF16),
            "wo": np.ascontiguousarray(np.asarray(Wo)[sl, :]).astype(BF16),
            "bqt": np.ascontiguousarray(
                np.asarray(bq, np.float32)[sl].reshape(OT, 128).T),
            "bkt": np.ascontiguousarray(
                np.asarray(bk, np.float32)[sl].reshape(OT, 128).T),
            "bvr": np.asarray(bv, np.float32)[sl].reshape(1, HG_D).astype(BF16),
            "bor": (np.asarray(bo, np.float32) if hg == 0
                    else np.zeros(D, np.float32)).reshape(1, D).astype(BF16),
        })
    return in_maps


def run_cores(in_maps, trace=False, with_bq=True, with_bk=True,
              with_bv=True, with_bo=True):
    try:
        import ntff_shim
        ntff_shim.install()
    except Exception:
        pass
    from concourse.bass_utils import run_bass_kernel_spmd

    nc = get_nc(with_bq, with_bk, with_bv, with_bo)
    return run_bass_kernel_spmd(nc, in_maps, list(range(N_CORES)), trace=trace)


def combine(results):
    y = np.empty((4, S, D), np.float32)
    for b in range(4):
        y[b] = (results[2 * b]["y"].astype(np.float32)
                + results[2 * b + 1]["y"].astype(np.float32))
    return y


def kernel(x, Wq, bq, Wk, bk, Wv, bv, Wo, bo):
    in_maps = make_in_maps(x, Wq, bq, Wk, bk, Wv, bv, Wo, bo)
    flags = dict(
        with_bq=bool(np.any(np.asarray(bq))),
        with_bk=bool(np.any(np.asarray(bk))),
        with_bv=bool(np.any(np.asarray(bv))),
        with_bo=bool(np.any(np.asarray(bo))),
    )
    res = run_cores(in_maps, trace=False, **flags)
    return combine(res.results)


# revision 20
# speedup vs baseline: 1.1739x; 1.1739x over previous
"""Multi-head attention (B=4, S=2048, D=1024, H=16, Hd=64) on 8 trn2 cores.

Sharding: core c = (batch b = c // 2, head-group hg = c % 2). Each core
computes attention for 8 heads of one batch and the corresponding slice of
the output projection; host sums the two partial outputs per batch.

Schedule model (per core, all matmuls bf16 with fp32 PSUM accumulation):
  PE pure work ~285us, ScalarE exp ~275us (256 x [128,1024] exps incl
  per-inst sync) -- both engines must stay saturated and overlapped.
  - DMA is issued column-blocked (xt by 512-wide q blocks interleaved with
    the weight slices the prologue needs first) so the first projection
    matmul starts ~3us in, not after the full 4MB xt load.
  - Prologue: kT p0 (4 groups) + qT(p0,c0)/(p1,c0); everything else (kT
    p1-p3, qT, v half1, o-proj) is filler work drained adaptively: every
    chunk iteration force-emits deadline-due fillers and then spends a
    budget of remaining_filler_cost/remaining_iterations, so the PE never
    starves while ScalarE paces the exp stream.
  - v projection is computed in half-groups (heads 0-3 / 4-7, N=256):
    half0 just-in-time inside chunk 0 (attnV[i] consumes v[st=i]), half1
    via fillers before the first p2 chunk.
  - Scores pair per (i): two K=64 matmuls on disjoint PE row halves run
    concurrently (~216ns/pair); attnV rides the denominator as a 65th
    output row; normalization is deferred one chunk and applied via K=1
    ones-matmul broadcasts + a DVE multiply.
  - PSUM: dedicated pools -- stt (scores+exp) 2 x [128,1024] (4 banks),
    otA/otB 2 x [65,512] (2 banks), proj/norm 2 x [128,512] (2 banks).

Per-core layout:
  xt   = x[b].T                    [D=1024, S=2048]  (lhsT/rhs K-major)
  qT/kT = (Wslice.T @ ..)          [512, 2048]  d-major, 4 pair-tiles of 128
  v    = x @ Wv_slice              [2048, 512]  s-major (+ones col per head)
  per head-pair chunk: scoresT[k,q] (row-tiled K=64 pair) -> exp
            outT[d,q] += v-block.T @ expT ; denom[q] += ones row
  y = outT.T-blocks @ Wo_slice + bo   [2048, 1024] bf16 partial
"""

import numpy as np
import ml_dtypes

S = 2048
D = 1024
HG_D = 512          # head dims per core (8 heads x 64)
NH = 8              # heads per core
KT = S // 128       # 16 k-tiles
DT = D // 128       # 8 contraction tiles for QKV
ST = S // 128       # 16 s-tiles
OT = HG_D // 128    # 4 contraction tiles for O-proj / pair tiles
N_CORES = 8

BF16 = ml_dtypes.bfloat16

_CACHED_NC = {}

# Chunk order (p = head-pair tile, c = 512-wide q chunk): all p0/p1 chunks
# first, then p2/p3 — so only v-half0 + kT p0/p1 are needed early and the
# p2/p3 prerequisites (v-half1, kT p2/p3) spread as budget fillers across
# eight chunks instead of being deadline-crammed into chunks 1-3.
CHUNK_ORDER = ([(p, c) for c in range(4) for p in (0, 1)]
               + [(p, c) for c in range(4) for p in (2, 3)])


def _build_nc(with_bq=True, with_bk=True, with_bv=True, with_bo=True):
    import concourse.bass as bass  # noqa: F401
    import concourse.mybir as mybir
    import concourse.tile as tile
    from concourse import bacc

    f32 = mybir.dt.float32
    bf16 = mybir.dt.bfloat16
    Exp = mybir.ActivationFunctionType.Exp

    nc = bacc.Bacc("TRN2", target_bir_lowering=False, debug=False,
                   num_devices=N_CORES)

    xt_d = nc.dram_tensor("xt", [D, S], bf16, kind="ExternalInput")
    wq_d = nc.dram_tensor("wq", [D, HG_D], bf16, kind="ExternalInput")
    wk_d = nc.dram_tensor("wk", [D, HG_D], bf16, kind="ExternalInput")
    wv_d = nc.dram_tensor("wv", [D, HG_D], bf16, kind="ExternalInput")
    wo_d = nc.dram_tensor("wo", [HG_D, D], bf16, kind="ExternalInput")
    bq_d = nc.dram_tensor("bqt", [128, OT], f32, kind="ExternalInput")
    bk_d = nc.dram_tensor("bkt", [128, OT], f32, kind="ExternalInput")
    bv_d = nc.dram_tensor("bvr", [1, HG_D], bf16, kind="ExternalInput")
    bo_d = nc.dram_tensor("bor", [1, D], bf16, kind="ExternalInput")
    y_d = nc.dram_tensor("y", [S, D], bf16, kind="ExternalOutput")

    with tile.TileContext(nc) as tc:
        with (
            tc.tile_pool(name="cpool", bufs=1) as cpool,
            tc.tile_pool(name="wpool", bufs=2) as wpool,
            tc.tile_pool(name="sttpool", bufs=2, space="PSUM") as sttpool,
            tc.tile_pool(name="pjpool", bufs=2, space="PSUM") as pjpool,
            tc.tile_pool(name="popool", bufs=2, space="PSUM") as popool,
        ):
            # ---- persistent SBUF tiles ----
            xt_sb = cpool.tile([128, DT, S], bf16, name="xt_sb")
            wq_sb = cpool.tile([128, DT, HG_D], bf16, name="wq_sb")
            wk_sb = cpool.tile([128, DT, HG_D], bf16, name="wk_sb")
            wv_sb = cpool.tile([128, DT, HG_D], bf16, name="wv_sb")
            wo_sb = cpool.tile([128, OT, D], bf16, name="wo_sb")
            bq_sb = cpool.tile([128, OT], f32, name="bq_sb")
            bk_sb = cpool.tile([128, OT], f32, name="bk_sb")
            bvr_sb = cpool.tile([1, HG_D], bf16, name="bvr_sb")
            bor_sb = cpool.tile([1, D], bf16, name="bor_sb")
            ones_t = cpool.tile([128, 128], bf16, name="ones_t")
            qT_sb = cpool.tile([128, OT, S], bf16, name="qT_sb")
            kT_sb = cpool.tile([128, OT, S], bf16, name="kT_sb")
            # v with a trailing ones column per head: attnv lhsT [128, 65]
            # whose 65th output row accumulates the softmax denominator.
            v_sb = cpool.tile([128, ST, NH, 65], bf16, name="v_sb")
            aoT_sb = cpool.tile([128, OT, S], bf16, name="aoT_sb")

            # ---- loads ----
            # One multi-dim descriptor per tensor piece (descriptor issue on
            # the Sync queue costs ~650ns each, so batching matters), ordered
            # so the prologue's first consumers unblock earliest: the first
            # kT group needs wk + xt q-block 0 only.
            xt_r = xt_d.rearrange("(k p) s -> p k s", p=128)
            nc.sync.dma_start(out=wk_sb[:], in_=wk_d.rearrange(
                "(k p) c -> p k c", p=128))
            nc.sync.dma_start(out=xt_sb[:, :, 0:512], in_=xt_r[:, :, 0:512])
            nc.sync.dma_start(out=wq_sb[:], in_=wq_d.rearrange(
                "(k p) c -> p k c", p=128))
            nc.sync.dma_start(out=wv_sb[:], in_=wv_d.rearrange(
                "(k p) c -> p k c", p=128))
            nc.sync.dma_start(out=xt_sb[:, :, 512:1024],
                              in_=xt_r[:, :, 512:1024])
            nc.sync.dma_start(out=xt_sb[:, :, 1024:2048],
                              in_=xt_r[:, :, 1024:2048])
            nc.sync.dma_start(out=wo_sb[:], in_=wo_d.rearrange(
                "(k p) c -> p k c", p=128))
            if with_bq:
                nc.sync.dma_start(out=bq_sb[:], in_=bq_d[:])
            if with_bk:
                nc.sync.dma_start(out=bk_sb[:], in_=bk_d[:])
            if with_bv:
                nc.sync.dma_start(out=bvr_sb[:], in_=bv_d[:])
            if with_bo:
                nc.sync.dma_start(out=bor_sb[:], in_=bo_d[:])
            nc.gpsimd.memset(ones_t[:], 1.0)
            # only the per-head trailing ones-column needs the constant
            nc.vector.memset(v_sb[:, :, :, 64:65], 1.0)

            # ---- projection group emitters ----
            def emit_qk_group(which, p, jc, on_scalar, half=None):
                if which == "q":
                    w_sb, b_sb, out_sb, wb = wq_sb, bq_sb, qT_sb, with_bq
                else:
                    w_sb, b_sb, out_sb, wb = wk_sb, bk_sb, kT_sb, with_bk
                lo = jc * 512 + (0 if not half else 256)
                w = 512 if half is None else 256
                pq = pjpool.tile([128, w], f32, tag="ps", name="pq")
                for k in range(DT):
                    nc.tensor.matmul(
                        pq[:],
                        w_sb[:, k, p * 128:(p + 1) * 128],
                        xt_sb[:, k, lo:lo + w],
                        start=(k == 0), stop=(k == DT - 1),
                    )
                dst = out_sb[:, p, lo:lo + w]
                if wb:
                    nc.scalar.add(dst, pq[:], b_sb[:, p:p + 1])
                elif on_scalar:
                    nc.scalar.copy(dst, pq[:])
                else:
                    nc.vector.tensor_copy(dst, pq[:])

            def emit_v_half(st, half):
                # heads 4*half..4*half+3, i.e. wv cols [256*half, 256*half+256)
                lo = 256 * half
                pv = pjpool.tile([128, 256], f32, tag="ps", name="pv")
                for k in range(DT):
                    nc.tensor.matmul(
                        pv[:],
                        xt_sb[:, k, st * 128:(st + 1) * 128],
                        wv_sb[:, k, lo:lo + 256],
                        start=(k == 0), stop=(not with_bv and k == DT - 1),
                    )
                if with_bv:
                    nc.tensor.matmul(pv[:], ones_t[0:1, 0:128],
                                     bvr_sb[0:1, lo:lo + 256],
                                     start=False, stop=True)
                nc.vector.tensor_copy(
                    v_sb[:, st, 4 * half:4 * half + 4, 0:64],
                    pv.rearrange("p (h c) -> p h c", c=64))

            def emit_oproj_half(st, l, on_scalar=False):
                yt = wpool.tile([128, 512], bf16, tag="y", bufs=3, name="yt")
                py = pjpool.tile([128, 512], f32, tag="ps", name="py")
                for kt in range(OT):
                    nc.tensor.matmul(
                        py[:],
                        aoT_sb[:, kt, st * 128:(st + 1) * 128],
                        wo_sb[:, kt, l * 512:(l + 1) * 512],
                        start=(kt == 0),
                        stop=(not with_bo and kt == OT - 1),
                    )
                if with_bo:
                    nc.tensor.matmul(py[:], ones_t[0:1, 0:128],
                                     bor_sb[0:1, l * 512:(l + 1) * 512],
                                     start=False, stop=True)
                if on_scalar:
                    nc.scalar.copy(yt[:], py[:])
                else:
                    nc.vector.tensor_copy(yt[:], py[:])
                nc.sync.dma_start(
                    out=y_d[st * 128:(st + 1) * 128, l * 512:(l + 1) * 512],
                    in_=yt[:])

            # ---- scheduling priority bands ----
            # Band 0: scores+exp (the exp cadence is the kernel clock).
            # Band 1: attnV + JIT v halves.
            # Band 2: fillers (kT/qT/v prereqs, o-proj backfill, norm) —
            #         above the chunk-end drain so a due filler's PSUM->SBUF
            #         copy isn't stuck behind the aoT/den/recip chain.
            # Band 3: chunk-end drain (aoT casts, denominator, reciprocal).
            import contextlib

            band_counters = [0, 10_000_000, 20_000_000, 30_000_000]

            @contextlib.contextmanager
            def band(n):
                saved = tc.cur_priority
                tc.cur_priority = band_counters[n]
                try:
                    yield
                finally:
                    band_counters[n] = tc.cur_priority
                    tc.cur_priority = saved

            # ---- deferred normalization ----
            pending = []
            norm_done = {c: 0 for c in range(4)}
            oproj_emitted = set()

            # ---- filler queue: (deadline, cost_ns, item) in deadline order.
            # item kinds: ("k",p,jc,half) ("q",p,jc,half) ("v",st,half)
            #             ("o",st,l)
            idx_of = {pc: i for i, pc in enumerate(CHUNK_ORDER)}
            first_p_idx = {p: min(i for (pp, _), i in idx_of.items()
                                  if pp == p) for p in range(4)}
            # Bulk p2/p3 prerequisites get SHAPED deadlines spread across
            # chunks 2..7 (well before their true need at chunk 8/9) so the
            # per-chunk forced amount stays ~2-3us instead of cramming the
            # p2-transition chunk.
            fillers = []
            for p in (1, 2, 3):
                for jc in range(4):
                    if p == 1 and jc < 2:
                        continue  # in prologue
                    for h in (0, 1):
                        if p == 1:
                            dl = first_p_idx[p] + jc / 4
                        else:
                            dl = 2.0 + (p - 2) * 2 + jc / 2 + h / 4
                        fillers.append((min(dl, first_p_idx[p] + jc / 4), 872,
                                        ("k", p, jc, h)))
            for st in range(ST):
                dl = min(2.0 + 6.0 * st / 16, first_p_idx[2] + st / 16)
                fillers.append((dl, 872, ("v", st, 1)))
            for idx, (p, c) in enumerate(CHUNK_ORDER):
                if idx >= 2:
                    for h in (0, 1):
                        fillers.append((idx - 0.3, 872, ("q", p, c, h)))
            fillers.sort(key=lambda f: f[0])
            rem_cost = [sum(f[1] for f in fillers)]

            def emit_filler(item):
                kind = item[0]
                with band(2):
                    if kind == "k" or kind == "q":
                        emit_qk_group(item[0], item[1], item[2],
                                      on_scalar=False, half=item[3])
                    elif kind == "v":
                        emit_v_half(item[1], item[2])
                    else:
                        emit_oproj_half(item[1], item[2])

            def add_oproj(c):
                for st in range(4 * c, 4 * c + 4):
                    if st not in oproj_emitted:
                        oproj_emitted.add(st)
                        for l in range(2):
                            fillers.append((float("inf"), 864, ("o", st, l)))
                            rem_cost[0] += 864

            def flush_normalize():
                with band(2):
                    _flush_normalize()

            def _flush_normalize():
                off2, p2, c2, rb2, row = pending.pop(0)
                off3, p3, c3, rb3, row3 = pending.pop(0)
                assert p2 == p3 and c2 == c3
                jb = slice(c2 * 512, (c2 + 1) * 512)
                bt = pjpool.tile([128, 512], f32, tag="ps", name="bt")
                nc.tensor.matmul(
                    bt[off2:off2 + 64, :],
                    ones_t[row:row + 1, 0:64],
                    rb2[row:row + 1, :],
                    start=True, stop=True,
                )
                nc.tensor.matmul(
                    bt[off3:off3 + 64, :],
                    ones_t[row3:row3 + 1, 0:64],
                    rb3[row3:row3 + 1, :],
                    start=True, stop=True,
                )
                nc.vector.tensor_mul(
                    aoT_sb[:, p2, jb],
                    aoT_sb[:, p2, jb],
                    bt[:])
                norm_done[c2] += 1
                if norm_done[c2] == OT:
                    add_oproj(c2)

            # drain fillers: everything past-due first (those are needed by
            # this very iteration's consumers), then optional budget work,
            # paced by remaining supply and capped by the iteration's PE
            # capacity so work never bunches. Forced items don't poison the
            # budget accumulator.
            iters_left = [len(CHUNK_ORDER) * KT]
            debt = [0.0]
            ITER_CAP = 1150.0

            def drain_fillers(now, base):
                work = base
                while fillers and fillers[0][0] <= now:
                    _, cost, item = fillers.pop(0)
                    rem_cost[0] -= cost
                    work += cost
                    emit_filler(item)
                debt[0] = min(debt[0] + rem_cost[0] / max(iters_left[0], 1),
                              1744.0)
                while (fillers and debt[0] > 0
                       and work + fillers[0][1] <= ITER_CAP + 400):
                    _, cost, item = fillers.pop(0)
                    rem_cost[0] -= cost
                    debt[0] -= cost
                    work += cost
                    emit_filler(item)

            # ---- attention chunk ----
            def emit_chunk(idx, p, c):
                jb = slice(c * 512, (c + 1) * 512)
                otA = popool.tile([65, 512], f32, tag="po", name="otA")
                otB = popool.tile([65, 512], f32, tag="po", name="otB")
                for i in range(KT):
                    # fillers FIRST: deadline-due producers (kT/qT/v read by
                    # this very iteration) must precede their consumers in
                    # program order, plus the adaptive budget share.
                    drain_fillers(idx + i / KT,
                                  base=647.0 + (872.0 if idx == 0 else 0.0))
                    if idx == 0:
                        with band(1):
                            emit_v_half(i, 0)
                    with band(0):
                        stt = sttpool.tile([128, 1024], f32, tag="stt",
                                           name="stt")
                        for off in (0, 64):
                            nc.tensor.matmul(
                                stt[:, off * 8:off * 8 + 512],
                                kT_sb[off:off + 64, p, i * 128:(i + 1) * 128],
                                qT_sb[off:off + 64, p, jb],
                                start=True, stop=True,
                            )
                        et = wpool.tile([128, 1024], bf16, tag="exp", bufs=8,
                                        name="et")
                        nc.scalar.activation(et[:], stt[:], Exp, scale=0.125)
                    with band(1):
                        for ot, hh in ((otA, 0), (otB, 1)):
                            nc.tensor.matmul(
                                ot[:],
                                v_sb[:, i, 2 * p + hh, :],
                                et[:, hh * 512:(hh + 1) * 512],
                                start=(i == 0), stop=(i == KT - 1),
                            )
                    if i == 15:
                        # flush the previous chunk's pair (its reciprocal
                        # finished ~a chunk ago) so o-proj unlocks early
                        while len(pending) > 0:
                            flush_normalize()
                    iters_left[0] -= 1
                # Drain attn rows first so the PSUM tiles free fast, then
                # gather the denominator rows and batch-reciprocal. Band 3:
                # this chain must not block due fillers' PSUM->SBUF copies
                # on the Vector queue at chunk boundaries.
                ctx = band(3)
                ctx.__enter__()
                nc.vector.tensor_copy(aoT_sb[0:64, p, jb], otA[0:64, :])
                nc.vector.tensor_copy(aoT_sb[64:128, p, jb], otB[0:64, :])
                den = wpool.tile([33, 512], f32, tag="den", bufs=2,
                                 name="den")
                nc.vector.tensor_copy(den[0:1, :], otA[64:65, :])
                nc.vector.tensor_copy(den[32:33, :], otB[64:65, :])
                rf = wpool.tile([33, 512], f32, tag="rf", name="rf")
                rb = wpool.tile([65, 512], bf16, tag="rb", bufs=3,
                                name="rb")
                # approx_fast needs a partition-0-based AP; rows 1..31 hold
                # stale values (harmless, unread). ~5x faster than
                # reciprocal(), ~3e-6 rel err.
                nc.vector.reciprocal_approx_fast(rf[0:33, :], den[0:33, :])
                nc.vector.tensor_copy(rb[64:65, :], rf[0:1, :])
                nc.vector.tensor_copy(rb[32:33, :], rf[32:33, :])
                pending.append((0, p, c, rb, 64))
                pending.append((64, p, c, rb, 32))
                ctx.__exit__(None, None, None)
                if p == 3:
                    # column-final chunk: flush immediately so this column's
                    # o-projection unlocks a full chunk earlier
                    while pending:
                        flush_normalize()

            # ---- prologue (ScalarE is idle before the first exp) ----
            # qT right after the first kT group: when the xt DMA completes it
            # unblocks everything at once and the scheduler follows emission
            # order, so the first chunk's q must come before the other kTs.
            # kT p1 jc0/jc1 ride here too (the prologue is DMA-paced and has
            # PE slack), halving chunk 1's deadline burst.
            emit_qk_group("k", 0, 0, on_scalar=True)
            emit_qk_group("q", *CHUNK_ORDER[0], on_scalar=True)
            emit_qk_group("q", *CHUNK_ORDER[1], on_scalar=True)
            for jc in range(1, 4):
                emit_qk_group("k", 0, jc, on_scalar=True)
            emit_qk_group("k", 1, 0, on_scalar=True)
            emit_qk_group("k", 1, 1, on_scalar=True)

            # ---- main schedule ----
            for idx, (p, c) in enumerate(CHUNK_ORDER):
                emit_chunk(idx, p, c)

            # ---- tail: flush the last normalizations + remaining o-proj ----
            while pending:
                flush_normalize()
            n = 0
            rest = [f[2] for f in fillers]
            fillers.clear()
            for item in rest:
                if item[0] == "o":
                    emit_oproj_half(item[1], item[2], on_scalar=(n % 2 == 1))
                    n += 1
                else:
                    emit_filler(item)
            for st in range(ST):
                if st not in oproj_emitted:
                    oproj_emitted.add(st)
                    for l in range(2):
                        emit_oproj_half(st, l, on_scalar=(n % 2 == 1))
                        n += 1

    nc.compile()
    return nc


def get_nc(with_bq=True, with_bk=True, with_bv=True, with_bo=True):
    key = (with_bq, with_bk, with_bv, with_bo)
    if key not in _CACHED_NC:
        _CACHED_NC[key] = _build_nc(*key)
    return _CACHED_NC[key]


def make_in_maps(x, Wq, bq, Wk, bk, Wv, bv, Wo, bo):
    x = np.asarray(x, dtype=np.float32)
    in_maps = []
    for c in range(N_CORES):
        b, hg = c // 2, c % 2
        sl = slice(hg * HG_D, (hg + 1) * HG_D)
        in_maps.append({
            "xt": np.ascontiguousarray(np.asarray(x[b]).T).astype(BF16),
            "wq": np.ascontiguousarray(np.asarray(Wq)[:, sl]).astype(BF16),
            "wk": np.ascontiguousarray(np.asarray(Wk)[:, sl]).astype(BF16),
            "wv": np.ascontiguousarray(np.asarray(Wv)[:, sl]).astype(BF16),
            "wo": np.ascontiguousarray(np.asarray(Wo)[sl, :]).astype(BF16),
            "bqt": np.ascontiguousarray(
                np.asarray(bq, np.float32)[sl].reshape(OT, 128).T),
            "bkt": np.ascontiguousarray(
                np.asarray(bk, np.float32)[sl].reshape(OT, 128).T),
            "bvr": np.asarray(bv, np.float32)[sl].reshape(1, HG_D).astype(BF16),
            "bor": (np.asarray(bo, np.float32) if hg == 0
                    else np.zeros(D, np.float32)).reshape(1, D).astype(BF16),
        })
    return in_maps


def run_cores(in_maps, trace=False, with_bq=True, with_bk=True,
              with_bv=True, with_bo=True):
    try:
        import ntff_shim
        ntff_shim.install()
    except Exception:
        pass
    from concourse.bass_utils import run_bass_kernel_spmd

    nc = get_nc(with_bq, with_bk, with_bv, with_bo)
    return run_bass_kernel_spmd(nc, in_maps, list(range(N_CORES)), trace=trace)


def combine(results):
    y = np.empty((4, S, D), np.float32)
    for b in range(4):
        y[b] = (results[2 * b]["y"].astype(np.float32)
                + results[2 * b + 1]["y"].astype(np.float32))
    return y


def kernel(x, Wq, bq, Wk, bk, Wv, bv, Wo, bo):
    in_maps = make_in_maps(x, Wq, bq, Wk, bk, Wv, bv, Wo, bo)
    flags = dict(
        with_bq=bool(np.any(np.asarray(bq))),
        with_bk=bool(np.any(np.asarray(bk))),
        with_bv=bool(np.any(np.asarray(bv))),
        with_bo=bool(np.any(np.asarray(bo))),
    )
    res = run_cores(in_maps, trace=False, **flags)
    return combine(res.results)
